# revision 51
# baseline (speedup 1.0000x reference)
"""Trainium2 Bass kernel: attention with LayerNorm on scores (sparse_attention).

Per-core work (1 of 8 heads, data-parallel over batch axis n):
    S   = (Q @ K^T) * 1/sqrt(d)          [L, L]
    Sn  = LayerNorm(S, axis=-1)          (gamma=1, beta=0 fast path)
    A   = softmax(Sn, axis=-1)
    out = A @ V                           [L, D]

Fast path (trivial affine) is K-MAJOR:
  - LayerNorm stats computed algebraically: G = K^T [K|1] once, H_t = Q_t G
    gives per-query sum-of-squares via rowsum(H ∘ Q) and mean via the ksum
    column -> per-query a_q = rstd_q / sqrt(d). The per-query SHIFT
    (-mu*rstd) cancels in softmax, so it is dropped entirely.
  - a_q is folded into Q (Qs = a ⊙ Q), then S~^T = K @ Qs^T is computed
    directly in k-major orientation: S^T tiles [128 k, q] in PSUM.
  - exp on ACT reads the PSUM tile and writes attnT [128 k, q] fp16 in SBUF
    -- ALREADY key-major for PV; no attention-matrix transpose anywhere.
  - PV: out^T += V_ct^T @ attnT_ct accumulated over k-tiles (V used in its
    natural layout as lhsT). Softmax denominator: attnT tiles are summed on
    DVE into acc, then matmul(lhsT=acc_chunk, rhs=ones) emits den directly
    in q-natural layout (the PE transposes acc as the stationary operand).
  - All main-loop matmuls are fp16 (1 cyc/row; fp32r measures ~4 cyc/row).
  - PE stream is software-pipelined: QK for k-tile ct+1 issues before PV for
    ct, so the PE never stalls behind the ACT exp (engine FIFOs are strict).
  - Loads ride one HWDGE queue per tensor with partition p taking 16
    contiguous DRAM rows (8KB descriptors). That permutes tiles to q%16 /
    k%16 groups: harmless -- k is a reduction axis, and the q permutation
    is undone by the output store's strided DRAM AP.
  - Q/Qs d-major transposes via DMA xbar in two half-tiles on the two HWDGE
    queues (two xbar writers on ONE tile race!). Stats run half-pipelined so
    block 0 of the main loop starts while half 1 still preps.
  - rstd via Sqrt (set pre-warmed at kernel start) + DVE reciprocal; Ln/Exp
    would ping-pong ACT table sets (~1.3us per switch).
"""

import numpy as np
from contextlib import ExitStack

import concourse.bass as bass
import concourse.bacc as bacc
import concourse.tile as tile
from concourse import mybir
from concourse import bass_utils
from concourse.masks import make_identity

F32 = mybir.dt.float32
F32R = mybir.dt.float32r
BF16 = mybir.dt.bfloat16
FP16 = mybir.dt.float16
AF = mybir.ActivationFunctionType
ALU = mybir.AluOpType

EPS = 1e-5
N_CORES = 8


def build_kernel_kmajor(L=2048, D=128, att_dt=FP16):
    """Trivial-affine fast path: k-major scores, no attn transpose."""
    P = 128
    T = L // P                  # 16 k-tiles / q-tiles
    QB = 1024                   # q columns per block
    NB = L // QB                # 2 blocks
    HQ = QB // 512              # 512-wide matmul chunks per block
    TB = QB // P                # q-tiles per block
    scale = 1.0 / np.sqrt(np.float32(D))

    nc = bacc.Bacc(
        "TRN2",
        target_bir_lowering=False,
        debug=False,
        enable_asserts=False,
        num_devices=N_CORES,
    )
    q_d = nc.dram_tensor("q", [L, D], F32, kind="ExternalInput").ap()
    k_d = nc.dram_tensor("k", [L, D], F32, kind="ExternalInput").ap()
    v_d = nc.dram_tensor("v", [L, D], F32, kind="ExternalInput").ap()
    out_d = nc.dram_tensor("out", [L, D], F32, kind="ExternalOutput").ap()

    with tile.TileContext(nc) as tc, ExitStack() as ctx:
        consts = ctx.enter_context(tc.tile_pool(name="consts", bufs=1))
        small = ctx.enter_context(tc.tile_pool(name="small", bufs=4))

        # ---- persistent SBUF tensors -----------------------------------
        q_sb = consts.tile([P, T, D], F32)          # natural Q (fp32)
        k_sb = consts.tile([P, T, D], F32)
        v_sb = consts.tile([P, T, D], F32)
        # Q-side tensors come in half-tiles (a: q-tiles 0..T/2, b: rest) so
        # the two DMA xbar transposes of each can run on both HWDGE queues
        # concurrently (two transposes must NOT write one tile: races).
        q16a = consts.tile([P, L // 2], att_dt)     # natural Q fp16, half 0
        q16b = consts.tile([P, L // 2], att_dt)
        qs16a = consts.tile([P, L // 2], att_dt)    # a ⊙ Q fp16
        qs16b = consts.tile([P, L // 2], att_dt)
        k16 = consts.tile([P, L], att_dt)           # natural K fp16 (contig)
        k16a = consts.tile([P, T, D + 2], att_dt)   # natural K fp16 + ones col
        v16 = consts.tile([P, T, D], att_dt)
        qT16a = consts.tile([P, L // 2], att_dt)    # Q^T (d-major) unscaled
        qT16b = consts.tile([P, L // 2], att_dt)
        qsT16a = consts.tile([P, L // 2], att_dt)   # (a ⊙ Q)^T
        qsT16b = consts.tile([P, L // 2], att_dt)
        kT16 = consts.tile([P, L], att_dt)          # K^T (d-major)
        g16 = consts.tile([P, D + 1], att_dt)       # G = K^T[K|1] fp16
        h_all = consts.tile([P, T, D + 1], F32)     # H tiles staged
        junk = consts.tile([P, T, D], F32)
        a_all = consts.tile([P, T], F32)            # per-query exp scale
        rden = consts.tile([P, T], F32)             # 1/softmax denominator
        den_nat = consts.tile([P, T], F32)          # den in q-natural layout
        attnT = consts.tile([P, T, QB], att_dt)     # exp(S^T) tiles, k-major
        oT32 = consts.tile([P, NB, QB], F32)        # PV result out^T fp32
        out_sb = consts.tile([P, T, D], F32)        # scaled out, staged layout
        ones16 = consts.tile([P, 1], att_dt)
        eps_t = consts.tile([P, 1], F32)
        eps2_t = consts.tile([P, 1], F32)
        sq_warm = consts.tile([P, 1], F32)
        ident_f = consts.tile([P, P], F32)

        nc.vector.memset(ones16, 1.0)
        nc.vector.memset(eps_t, EPS)
        nc.vector.memset(eps2_t, EPS * float(D))
        nc.gpsimd.memset(k16a[:, :, D : D + 2], 1.0)
        make_identity(nc, ident_f)
        # pre-warm the Sqrt ACT table set while everything waits on loads
        nc.scalar.activation(sq_warm, eps_t, AF.Sqrt)

        TH = T // 2
        q16av = q16a.rearrange("p (t d) -> p t d", d=D)
        q16bv = q16b.rearrange("p (t d) -> p t d", d=D)
        qs16av = qs16a.rearrange("p (t d) -> p t d", d=D)
        qs16bv = qs16b.rearrange("p (t d) -> p t d", d=D)
        qT16av = qT16a.rearrange("p (t q) -> p t q", q=P)
        qT16bv = qT16b.rearrange("p (t q) -> p t q", q=P)
        qsT16av = qsT16a.rearrange("p (t q) -> p t q", q=P)
        qsT16bv = qsT16b.rearrange("p (t q) -> p t q", q=P)
        k16v = k16.rearrange("p (t d) -> p t d", d=D)
        kT16v = kT16.rearrange("p (t q) -> p t q", q=P)

        # Bulk staged loads: partition p takes rows {16p..16p+15} -- 4-8KB
        # descriptors (full DMA bandwidth). This lands tiles in a mod-16
        # row-interleaved order; k/q tile identity is arbitrary (softmax
        # reduces over k; q order is undone by the output store AP). Each
        # HWDGE queue feeds ~half the DMA engines, so split q/k by column
        # halves across both queues; v rides the gpsimd SWDGE ring.
        qv_d = q_d.rearrange("(p c) d -> p c d", c=T)
        kv_d = k_d.rearrange("(p c) d -> p c d", c=T)
        nc.sync.dma_start(out=q_sb, in_=qv_d)
        nc.scalar.dma_start(out=k_sb, in_=kv_d)
        nc.gpsimd.dma_start(out=v_sb, in_=v_d.rearrange("(p c) d -> p c d", c=T))

        nc.scalar.copy(out=q16av, in_=q_sb[:, 0:TH, :])
        nc.vector.tensor_copy(q16bv, q_sb[:, TH:T, :])
        nc.vector.tensor_copy(k16a[:, :, 0:D], k_sb)   # gates G
        # d-major transposes via DMA xbar; half-a on sync, half-b on scalar
        nc.sync.dma_start_transpose(qT16av, q16a)
        nc.scalar.dma_start_transpose(qT16bv, q16b)
        nc.vector.tensor_copy(k16v, k_sb)              # gates kT16
        nc.scalar.dma_start_transpose(kT16v, k16)
        nc.scalar.copy(out=v16, in_=v_sb)              # gates PV only

        # ---- precompute: G, H, per-query scale -------------------------
        with tc.tile_pool(name="pre_ps", bufs=2, space="PSUM") as pre_ps, \
             tc.tile_pool(name="g_ps", bufs=1, space="PSUM") as g_psp:
            # G = sum_t K_t^T [K_t | 1]  ->  [D, D+1]  (fp16 inputs)
            g_ps = g_psp.tile([P, D + 1], F32, tag="ps_g")
            for t in range(T):
                nc.tensor.matmul(
                    g_ps,
                    lhsT=k16a[:, t, 0:D],
                    rhs=k16a[:, t, 0 : D + 1],
                    start=(t == 0),
                    stop=(t == T - 1),
                )
            nc.scalar.copy(out=g16, in_=g_ps)

            # H_t = Q_t G, then stats + scaled-Q, pipelined in tile-halves
            # so block 0 of the main loop starts while half 1 still preps.
            #   esq_t = rowsum(H_t[:, :D] ∘ Q_t) * scale^2 / L
            #   mu_t  = H_t[:, D] * scale / L ;  var = esq - mu^2
            #   rstd = exp(-0.5 ln(var+eps))   (Ln+Exp share one table set)
            esq = small.tile([P, T], F32, tag="esq")
            mu = small.tile([P, T], F32, tag="mu")
            var = small.tile([P, T], F32, tag="var")
            for hf in range(2):
                qT_h = qT16a if hf == 0 else qT16b
                qs16_h = qs16av if hf == 0 else qs16bv
                hs = slice(hf * TH, (hf + 1) * TH)
                for tp_ in range(TH // 2):
                    t0 = hf * TH + 2 * tp_
                    h_ps = pre_ps.tile([P, 2, D + 1], F32, tag="ps_h")
                    for u in range(2):
                        c0 = (2 * tp_ + u) * P
                        nc.tensor.matmul(
                            h_ps[:, u, :],
                            lhsT=qT_h[:, c0 : c0 + P],
                            rhs=g16,
                            start=True,
                            stop=True,
                        )
                    if tp_ % 2:
                        nc.scalar.copy(out=h_all[:, t0 : t0 + 2, :], in_=h_ps)
                    else:
                        nc.vector.tensor_copy(
                            out=h_all[:, t0 : t0 + 2, :], in_=h_ps
                        )
                nc.vector.tensor_tensor(
                    junk[:, hs, :], h_all[:, hs, 0:D], q_sb[:, hs, :],
                    op=ALU.mult,
                )
                nc.vector.tensor_reduce(
                    esq[:, hs], junk[:, hs, :], axis=mybir.AxisListType.X,
                    op=ALU.add,
                )
                nc.vector.tensor_scalar_mul(
                    esq[:, hs], esq[:, hs], float(scale) * float(scale) / L
                )
                nc.vector.tensor_scalar_mul(
                    mu[:, hs], h_all[:, hs, D : D + 1], float(scale) / L
                )
                nc.vector.tensor_tensor(
                    var[:, hs], mu[:, hs], mu[:, hs], op=ALU.mult
                )
                nc.vector.tensor_sub(var[:, hs], esq[:, hs], var[:, hs])
                # a = scale/sqrt(var+eps) = 1/sqrt(var/scale^2 + eps/scale^2)
                nc.scalar.activation(
                    var[:, hs], var[:, hs], AF.Sqrt,
                    bias=eps2_t, scale=float(D),
                )
                nc.vector.reciprocal(a_all[:, hs], var[:, hs])
                for i in range(TH):
                    t = hf * TH + i
                    nc.vector.tensor_scalar_mul(
                        qs16_h[:, i, :], q_sb[:, t, :], a_all[:, t : t + 1]
                    )
                # both qsT xbar issues live on the sync queue: the scalar
                # queue is about to stream the main-loop exps
                if hf == 0:
                    nc.sync.dma_start_transpose(qsT16av, qs16a)
                else:
                    nc.sync.dma_start_transpose(qsT16bv, qs16b)

        # ---- main loop: k-major S^T -> exp -> PV; den on DVE -----------
        with (
            tc.tile_pool(name="s_ps", bufs=2, space="PSUM") as s_psp,
            tc.tile_pool(name="o_ps", bufs=1, space="PSUM") as o_psp,
            tc.tile_pool(name="d_ps", bufs=1, space="PSUM") as d_psp,
            tc.tile_pool(name="tr_ps", bufs=1, space="PSUM") as tr_psp,
            tc.tile_pool(name="accp", bufs=2) as acc_pool,
        ):
            for qb in range(NB):
                o_ps = o_psp.tile([P, QB], F32, tag="o")
                acc = acc_pool.tile([P, QB], att_dt, tag="acc")

                def pv(ct, o_ps=o_ps):
                    for h in range(HQ):
                        hs = slice(h * 512, (h + 1) * 512)
                        nc.tensor.matmul(
                            o_ps[:, hs],
                            lhsT=v16[:, ct, :],
                            rhs=attnT[:, ct, hs],
                            start=(ct == 0),
                            stop=(ct == T - 1),
                        )

                qsT_blk = qsT16a if qb == 0 else qsT16b
                for ct in range(T):
                    s_ps = s_psp.tile([P, QB], F32, tag="s")
                    for h in range(HQ):
                        nc.tensor.matmul(
                            s_ps[:, h * 512 : (h + 1) * 512],
                            lhsT=kT16[:, ct * P : (ct + 1) * P],
                            rhs=qsT_blk[:, h * 512 : (h + 1) * 512],
                            start=True,
                            stop=True,
                        )
                    nc.scalar.activation(attnT[:, ct, :], s_ps, AF.Exp)
                    # running denominator sum on DVE (fp16)
                    if ct == 0:
                        nc.vector.tensor_copy(out=acc, in_=attnT[:, 0, :])
                    else:
                        nc.vector.tensor_tensor(
                            acc, acc, attnT[:, ct, :], op=ALU.add
                        )
                    # software pipeline: PV trails QK by one k-tile
                    if ct >= 1:
                        pv(ct - 1)

                pv(T - 1)
                # den in q-natural layout directly: den[:, i] = acc_chunk^T
                # @ ones -- the PE transposes acc as the stationary operand
                den_ps = d_psp.tile([P, TB], F32, tag="dn")
                for i in range(TB):
                    nc.tensor.matmul(
                        den_ps[:, i : i + 1],
                        lhsT=acc[:, i * P : (i + 1) * P],
                        rhs=ones16,
                        start=True,
                        stop=True,
                    )
                bt = slice(qb * TB, (qb + 1) * TB)
                nc.vector.tensor_copy(out=den_nat[:, bt], in_=den_ps)
                nc.vector.reciprocal(rden[:, bt], den_nat[:, bt])

                # per-block post: out^T -> SBUF -> PE transpose -> scale by
                # 1/den fused into the PSUM->SBUF copy
                nc.scalar.copy(out=oT32[:, qb, :], in_=o_ps)
                tr_ps = tr_psp.tile([P, 4, P], F32, tag="tr")
                out_v = out_d.rearrange("(p c) d -> p c d", c=T)
                for i in range(TB):
                    t = qb * TB + i
                    nc.tensor.transpose(
                        tr_ps[:, i % 4, :],
                        oT32[:, qb, i * P : (i + 1) * P],
                        ident_f,
                    )
                    nc.vector.tensor_scalar_mul(
                        out_sb[:, t, :], tr_ps[:, i % 4, :],
                        rden[:, t : t + 1],
                    )
                # one store per block: staged layout -> 4KB DRAM runs
                seng = nc.sync if qb == 0 else nc.scalar
                seng.dma_start(
                    out=out_v[:, qb * TB : (qb + 1) * TB, :],
                    in_=out_sb[:, qb * TB : (qb + 1) * TB, :],
                )

    nc.compile()
    return nc


def build_kernel_general(L=2048, D=128, qk_dt=F32R, att_dt=FP16):
    """Fallback (non-trivial gamma/beta): q-major with DMA attn transpose."""
    P = 128
    ATT_DT = att_dt
    T = L // P
    CH = min(512, L)
    NB = L // CH
    TPB = max(1, min(4, T))
    scale = 1.0 / np.sqrt(np.float32(D))

    nc = bacc.Bacc(
        "TRN2",
        target_bir_lowering=False,
        debug=False,
        enable_asserts=False,
        num_devices=N_CORES,
    )
    q_d = nc.dram_tensor("q", [L, D], F32, kind="ExternalInput").ap()
    k_d = nc.dram_tensor("k", [L, D], F32, kind="ExternalInput").ap()
    v_d = nc.dram_tensor("v", [L, D], F32, kind="ExternalInput").ap()
    g_d = nc.dram_tensor("gamma", [L], F32, kind="ExternalInput").ap()
    b_d = nc.dram_tensor("beta", [L], F32, kind="ExternalInput").ap()
    out_d = nc.dram_tensor("out", [L, D], F32, kind="ExternalOutput").ap()

    with tile.TileContext(nc) as tc, ExitStack() as ctx:
        consts = ctx.enter_context(tc.tile_pool(name="consts", bufs=1))
        small = ctx.enter_context(tc.tile_pool(name="small", bufs=4))

        q_sb = consts.tile([P, T, D], F32)
        k_aug = consts.tile([P, T, D + 4], F32)
        v_bf = consts.tile([P, T, D], ATT_DT)
        qT = consts.tile([P, L], F32)
        qT_r = consts.tile([P, L], qk_dt)
        kT_r = consts.tile([P, L], qk_dt)
        G_sb = consts.tile([P, D + 1], F32)
        a_all = consts.tile([P, T], F32)
        b_all = consts.tile([P, T], F32)
        rden_all = consts.tile([P, T], F32)
        ident_f = consts.tile([P, P], F32)
        eps_t = consts.tile([P, 1], F32)

        make_identity(nc, ident_f)
        nc.vector.memset(eps_t, EPS)
        nc.vector.memset(k_aug[:, :, D : D + 1], 1.0)

        gam_bc = consts.tile([P, L], F32)
        bet_bc = consts.tile([P, L], F32)
        nc.sync.dma_start(out=gam_bc, in_=g_d.to_broadcast((P, L)))
        nc.sync.dma_start(out=bet_bc, in_=b_d.to_broadcast((P, L)))

        v_sb = consts.tile([P, T, D], F32)
        for t in range(T):
            r = slice(t * P, (t + 1) * P)
            nc.sync.dma_start(out=q_sb[:, t, :], in_=q_d[r, :])
            nc.sync.dma_start(out=k_aug[:, t, 0:D], in_=k_d[r, :])
            nc.sync.dma_start(out=v_sb[:, t, :], in_=v_d[r, :])
        nc.vector.tensor_copy(v_bf, v_sb)

        with tc.tile_pool(name="pre_ps", bufs=2, space="PSUM") as pre_ps:
            for t in range(T):
                c = slice(t * P, (t + 1) * P)
                ps1 = pre_ps.tile([P, P], F32, tag="ps_tr")
                nc.tensor.transpose(ps1, q_sb[:, t, :], ident_f)
                nc.scalar.copy(out=qT[:, c], in_=ps1)
                nc.vector.tensor_copy(out=qT_r[:, c], in_=ps1)
                ps2 = pre_ps.tile([P, P], F32, tag="ps_tr")
                nc.tensor.transpose(ps2, k_aug[:, t, 0:D], ident_f)
                nc.scalar.copy(out=kT_r[:, c], in_=ps2)

            g_ps = pre_ps.tile([P, D + 1], F32, tag="ps_g")
            for t in range(T):
                nc.tensor.matmul(
                    g_ps,
                    lhsT=k_aug[:, t, 0:D],
                    rhs=k_aug[:, t, 0 : D + 1],
                    start=(t == 0),
                    stop=(t == T - 1),
                )
            nc.scalar.copy(out=G_sb, in_=g_ps)

            for t in range(T):
                c = slice(t * P, (t + 1) * P)
                h_ps = pre_ps.tile([P, D + 1], F32, tag="ps_h")
                nc.tensor.matmul(
                    h_ps, lhsT=qT[:, c], rhs=G_sb, start=True, stop=True
                )
                h_sb = small.tile([P, D + 1], F32, tag="hsb")
                nc.scalar.copy(out=h_sb, in_=h_ps)
                nmu = small.tile([P, 1], F32, tag="nmu")
                junk = small.tile([P, D], F32, tag="junk")
                esq = small.tile([P, 1], F32, tag="esq")
                var = small.tile([P, 1], F32, tag="var")
                rstd = small.tile([P, 1], F32, tag="rstd")
                nc.vector.tensor_scalar_mul(
                    nmu, h_sb[:, D : D + 1], -float(scale) / L
                )
                nc.vector.tensor_tensor(junk, h_sb[:, 0:D], q_sb[:, t, :], op=ALU.mult)
                nc.vector.tensor_reduce(esq, junk, axis=mybir.AxisListType.X, op=ALU.add)
                nc.vector.tensor_scalar_mul(esq, esq, float(scale) * float(scale) / L)
                nc.vector.tensor_tensor(var, nmu, nmu, op=ALU.mult)
                nc.vector.tensor_sub(var, esq, var)
                nc.scalar.activation(rstd, var, AF.Sqrt, bias=eps_t)
                nc.vector.reciprocal(rstd, rstd)
                nc.vector.tensor_scalar_mul(
                    a_all[:, t : t + 1], rstd, float(scale)
                )
                nc.vector.tensor_tensor(
                    b_all[:, t : t + 1], nmu, rstd, op=ALU.mult
                )

        with (
            tc.tile_pool(name="s_ps", bufs=4, space="PSUM") as s_psp,
            tc.tile_pool(name="tr_ps", bufs=2, space="PSUM") as tr_psp,
            tc.tile_pool(name="o_ps", bufs=2, space="PSUM") as o_psp,
            tc.tile_pool(name="attn", bufs=3) as attn_pool,
            tc.tile_pool(name="attnT", bufs=2) as pT_pool,
            tc.tile_pool(name="osb", bufs=2) as osb_pool,
            tc.tile_pool(name="outp", bufs=3) as out_pool,
        ):
            n_blk = (T + TPB - 1) // TPB
            for j in range(n_blk):
                blkq = TPB * P
                attnT = pT_pool.tile([P, TPB, T, P], ATT_DT)
                for tt in range(TPB):
                    t = j * TPB + tt
                    qc = slice(t * P, (t + 1) * P)
                    attn = attn_pool.tile([P, L], ATT_DT)
                    dacc = small.tile([P, NB], F32, tag="dacc")
                    for nb in range(NB):
                        kc = slice(nb * CH, (nb + 1) * CH)
                        s_ps = s_psp.tile([P, CH], F32, tag="s")
                        nc.tensor.matmul(
                            s_ps,
                            lhsT=qT_r[:, qc],
                            rhs=kT_r[:, kc],
                            start=True,
                            stop=True,
                        )
                        y = small.tile([P, CH], F32, tag="y")
                        nc.vector.tensor_scalar(
                            y,
                            in0=s_ps,
                            scalar1=a_all[:, t : t + 1],
                            scalar2=b_all[:, t : t + 1],
                            op0=ALU.mult,
                            op1=ALU.add,
                        )
                        nc.vector.tensor_tensor(
                            y, y, gam_bc[:, kc], op=ALU.mult
                        )
                        nc.vector.tensor_add(y, y, bet_bc[:, kc])
                        nc.scalar.activation(
                            attn[:, kc],
                            y,
                            AF.Exp,
                            accum_out=dacc[:, nb : nb + 1],
                        )
                    den = small.tile([P, 1], F32, tag="den")
                    nc.vector.tensor_reduce(
                        den, dacc, axis=mybir.AxisListType.X, op=ALU.add
                    )
                    nc.vector.reciprocal(rden_all[:, t : t + 1], den)
                    half = T // 2
                    for hh in range(2):
                        nc.sync.dma_start_transpose(
                            attnT[:, tt, hh * half : (hh + 1) * half, :],
                            attn[:, hh * half * P : (hh + 1) * half * P],
                        )
                o_ps = o_psp.tile([P, blkq], F32, tag="o")
                for ct in range(T):
                    nc.tensor.matmul(
                        o_ps,
                        lhsT=v_bf[:, ct, :],
                        rhs=attnT[:, :, ct, :],
                        start=(ct == 0),
                        stop=(ct == T - 1),
                    )
                oT = osb_pool.tile([P, blkq], F32)
                nc.scalar.copy(out=oT, in_=o_ps)
                for tt in range(TPB):
                    t = j * TPB + tt
                    tr2 = tr_psp.tile([P, P], F32, tag="tr")
                    nc.tensor.transpose(
                        tr2, oT[:, tt * P : (tt + 1) * P], ident_f
                    )
                    ot = out_pool.tile([P, D], F32)
                    nc.scalar.activation(
                        ot, tr2, AF.Copy, scale=rden_all[:, t : t + 1]
                    )
                    nc.sync.dma_start(
                        out=out_d[t * P : (t + 1) * P, :], in_=ot
                    )

    nc.compile()
    return nc


_CACHE = {}


def _get_nc(L, D, trivial):
    key = (L, D, trivial)
    if key not in _CACHE:
        if trivial:
            _CACHE[key] = build_kernel_kmajor(L, D)
        else:
            _CACHE[key] = build_kernel_general(L, D)
    return _CACHE[key]


def kernel(q, k, v, gamma, beta, _trace=False):
    n, L, D = q.shape
    assert n == N_CORES
    trivial = bool(np.all(gamma == 1.0) and np.all(beta == 0.0))
    nc = _get_nc(L, D, trivial)
    in_maps = []
    for c in range(n):
        m = {
            "q": np.ascontiguousarray(q[c], dtype=np.float32),
            "k": np.ascontiguousarray(k[c], dtype=np.float32),
            "v": np.ascontiguousarray(v[c], dtype=np.float32),
        }
        if not trivial:
            m["gamma"] = np.ascontiguousarray(gamma, dtype=np.float32)
            m["beta"] = np.ascontiguousarray(beta, dtype=np.float32)
        in_maps.append(m)
    res = bass_utils.run_bass_kernel_spmd(
        nc, in_maps, core_ids=list(range(n)), trace=_trace
    )
    out = np.stack([res.results[c]["out"] for c in range(n)], axis=0)
    if _trace:
        kernel.last_exec_time_ns = res.exec_time_ns
        kernel.last_results = res
    return out.astype(np.float32)
